# revision 1
# baseline (speedup 1.0000x reference)
"""GumbelVectorQuantizer eval-path kernel for 8 Trainium2 NeuronCores.

Strategy (per sharding hint): data-parallel over BT rows across the 8
cores; the projection W [768,640] and codebook [640,128] are replicated.
Each core computes logits = x_shard @ W + b, per-group argmax -> one-hot
gather of codebook rows (q), and the per-shard softmax-probability sums
for the perplexity diagnostic. The tiny [2,320] partials are reduced on
the host (cheaper than an all-reduce for 2.5 KB) and the perplexity
scalar is computed there in float64.

Shapes are hardcoded for the problem instance:
  x [16, 2048, 768] f32, W [768, 640] f32, b [640] f32,
  codebook [1, 640, 128] f32.
Outputs match reference(): (q [16,2048,256] f32, quantize_prob_ppl f32,
curr_temp f32).
"""

import functools

import numpy as np

GROUPS = 2
NUM_VARS = 320
VAR_DIM = 128
CURR_TEMP = 2.0
EPS = 1e-7
N_CORES = 8


@functools.lru_cache(maxsize=1)
def _compiled():
    import jax
    import jax.numpy as jnp

    devs = jax.devices()[:N_CORES]

    def shard_fn(x2d, W, b, cb):
        # x2d: [BT/8, 768]; W: [768, 640]; b: [640]; cb: [640, 128]
        logits = x2d @ W + b                                   # [R, G*V]
        lg = logits.reshape(-1, GROUPS, NUM_VARS)              # [R, G, V]
        k = jnp.argmax(lg, axis=-1)                            # [R, G]
        cbg = cb.reshape(GROUPS, NUM_VARS, VAR_DIM)
        # gather: exact codebook rows
        q = jnp.concatenate(
            [jnp.take(cbg[g], k[:, g], axis=0) for g in range(GROUPS)], axis=-1
        )                                                      # [R, G*D]
        m = jax.lax.stop_gradient(lg.max(axis=-1, keepdims=True))
        e = jnp.exp(lg - m)
        p = e / e.sum(axis=-1, keepdims=True)                  # [R, G, V]
        pp = p.sum(axis=0)                                     # [G, V]
        return q, pp

    pm = jax.pmap(shard_fn, devices=devs,
                  in_axes=(0, None, None, None))
    return jax, jnp, pm


def kernel(x, W, b, codebook):
    jax, jnp, pm = _compiled()
    x = np.asarray(x, dtype=np.float32)
    W = np.asarray(W, dtype=np.float32)
    b = np.asarray(b, dtype=np.float32)
    cb = np.asarray(codebook, dtype=np.float32).reshape(GROUPS * NUM_VARS, VAR_DIM)

    bsz, tsz, fsz = x.shape
    bt = bsz * tsz
    rows = bt // N_CORES
    xs = x.reshape(N_CORES, rows, fsz)

    q_sh, pp_sh = pm(xs, W, b, cb)
    q = np.asarray(q_sh).reshape(bsz, tsz, GROUPS * VAR_DIM)

    # host-side reduction of the tiny perplexity partials, in float64
    avg_probs = np.asarray(pp_sh, dtype=np.float64).sum(axis=0) / float(bt)  # [G, V]
    ppl = np.exp(-np.sum(avg_probs * np.log(avg_probs + EPS), axis=-1)).sum()
    total = GROUPS * NUM_VARS
    qppl = (total - ppl) / total

    return q, np.float32(qppl), np.float32(CURR_TEMP)
